# revision 5
# baseline (speedup 1.0000x reference)
"""CosineSimCodebook VQ kernel for 8 TRN2 NeuronCores.

Sharding:
  - dist/argmax/quantize: token-parallel (each core owns 1024 of 8192 tokens,
    computes dist against ALL 8192 codes -> local argmax is global argmax).
  - EMA update: code-parallel (each core owns 1024 of 8192 codes). The
    normalized tokens (xn, rounded to fp32r) and the argmax indices are
    AllGathered; each core then builds one-hot tiles for its code shard and
    accumulates embed_sum with a one-hot matmul over all tokens.

Precision:
  - dist matmul: host-split bf16 hi/lo 3-pass (x@e = xh@eh + xh@el + xl@eh),
    fp32 PSUM accumulation; row 1/||x|| scale fused into the PSUM->SBUF copy.
    Abs error ~1e-6 -- argmax matches fp32 reference.
  - embed_sum: one-hot (exact) fp32r matmul against fp32r xn (~3e-5 rel).
  - Laplace smoothing cancels inside l2norm (positive per-row scalar), so
    new_embed = new_embed_avg / max(||new_embed_avg||, 1e-6) directly.
"""
import sys

import numpy as np

if "/opt/trn_rl_repo" not in sys.path:
    sys.path.insert(0, "/opt/trn_rl_repo")

import ml_dtypes  # noqa: E402

import concourse.bass as bass  # noqa: E402
import concourse.tile as tile  # noqa: E402
from concourse import bacc, mybir  # noqa: E402
from concourse.bass_utils import run_bass_kernel_spmd  # noqa: E402

P = 128
D = 512           # embedding dim
T = 8192          # total tokens (4*2048)
C = 8192          # codebook size
NCORES = 8
TC = T // NCORES  # tokens per core (1024)
CC = C // NCORES  # codes per core (1024)
NB = 4            # dist code superblocks
BLK = C // NB     # 2048 codes per superblock
NTT = TC // P     # token tiles per core (8)
NTG = T // P      # global token tiles (64)
NCC = CC // P     # code chunks per core (8)
DECAY = 0.8
EPS_NORM = 1e-6

f32 = mybir.dt.float32
f32r = mybir.dt.float32r
bf16 = mybir.dt.bfloat16
i32 = mybir.dt.int32
u32 = mybir.dt.uint32
u8 = mybir.dt.uint8

_CACHE = {}


def _build():
    nc = bacc.Bacc("TRN2", target_bir_lowering=False, debug=False,
                   enable_asserts=False, num_devices=NCORES)

    # ---- I/O (per core) ----
    x_in = nc.dram_tensor("x_sh", [TC, D], f32, kind="ExternalInput").ap()
    xTh_in = nc.dram_tensor("xTh", [D, TC], bf16, kind="ExternalInput").ap()
    xTl_in = nc.dram_tensor("xTl", [D, TC], bf16, kind="ExternalInput").ap()
    eTh_in = nc.dram_tensor("eTh", [D, C], bf16, kind="ExternalInput").ap()
    eTl_in = nc.dram_tensor("eTl", [D, C], bf16, kind="ExternalInput").ap()
    emb_in = nc.dram_tensor("emb", [C, D], f32, kind="ExternalInput").ap()
    ea_in = nc.dram_tensor("ea_sh", [CC, D], f32, kind="ExternalInput").ap()
    cs_in = nc.dram_tensor("cs_sh", [1, CC], f32, kind="ExternalInput").ap()
    offs_in = nc.dram_tensor("offs", [P, 1], f32, kind="ExternalInput").ap()

    dist_out = nc.dram_tensor("dist_sh", [TC, C], f32, kind="ExternalOutput").ap()
    ind_out = nc.dram_tensor("ind_sh", [TC, 1], i32, kind="ExternalOutput").ap()
    qz_out = nc.dram_tensor("qz_sh", [TC, D], f32, kind="ExternalOutput").ap()
    ncs_out = nc.dram_tensor("ncs_sh", [1, CC], f32, kind="ExternalOutput").ap()
    nea_out = nc.dram_tensor("nea_sh", [CC, D], f32, kind="ExternalOutput").ap()
    nem_out = nc.dram_tensor("nem_sh", [CC, D], f32, kind="ExternalOutput").ap()

    AFT = mybir.ActivationFunctionType

    with tile.TileContext(nc) as tc:
        with (
            tc.tile_pool(name="cst", bufs=1) as cst,
            tc.tile_pool(name="xp", bufs=2) as xp,          # x / xn streaming
            tc.tile_pool(name="xtp", bufs=1) as xtp,        # resident xT hi/lo
            tc.tile_pool(name="etp", bufs=2) as etp,        # eT superblocks
            tc.tile_pool(name="dtp", bufs=3) as dtp,        # dist sbuf blocks
            tc.tile_pool(name="smp", bufs=1) as smp,        # small persistent
            tc.tile_pool(name="ph2", bufs=3) as ph2,        # phase-2 streaming
            tc.tile_pool(name="pp", bufs=2, space="PSUM") as pp,
            tc.tile_pool(name="esp", bufs=1, space="PSUM") as esp,
            tc.tile_pool(name="bnp", bufs=1, space="PSUM") as bnp,
            tc.tile_pool(name="dram", bufs=1, space="DRAM") as dram,
        ):
            # ---- constants ----
            iota_i = cst.tile([P, CC], i32)
            nc.gpsimd.iota(iota_i[:], pattern=[[1, CC]], base=0,
                           channel_multiplier=0)
            iota_f = cst.tile([P, CC], f32)
            nc.vector.tensor_copy(iota_f[:], iota_i[:])
            ones_f = cst.tile([P, 1], f32)
            nc.vector.memset(ones_f[:], 1.0)
            offs_t = cst.tile([P, 1], f32)
            nc.sync.dma_start(out=offs_t[:], in_=offs_in[:])

            # ---- collective buffers ----
            xn_bounce = dram.tile([TC, D], f32r, name="xn_bounce")
            xn_g = dram.tile([T, D], f32r, name="xn_g", addr_space="Shared")
            ind_bounce = dram.tile([TC, 1], i32, name="ind_bounce")
            ind_g = dram.tile([T, 1], i32, name="ind_g", addr_space="Shared")

            # ================= phase 0: norms + xn =================
            inv_t = smp.tile([P, NTT], f32)   # 1/max(||x_t||,eps), col=token tile
            for tt in range(NTT):
                x_t = xp.tile([P, D], f32, name="x_t", tag="x_t")
                nc.sync.dma_start(out=x_t[:], in_=x_in[tt * P:(tt + 1) * P, :])
                sq = xp.tile([P, D], f32, name="sq", tag="sq")
                ssq = xp.tile([P, 1], f32, name="ssq", tag="ssq")
                nc.scalar.activation(sq[:], x_t[:], AFT.Square, accum_out=ssq[:])
                nrm = xp.tile([P, 1], f32, name="nrm", tag="nrm")
                nc.scalar.sqrt(nrm[:], ssq[:])
                nc.vector.tensor_scalar_max(nrm[:], nrm[:], EPS_NORM)
                nc.vector.reciprocal(inv_t[:, tt:tt + 1], nrm[:])
                xn_t = xp.tile([P, D], f32r, name="xn_t", tag="xn_t")
                nc.scalar.mul(xn_t[:], x_t[:], inv_t[:, tt:tt + 1])
                nc.sync.dma_start(out=xn_bounce[tt * P:(tt + 1) * P, :],
                                  in_=xn_t[:])
            nc.gpsimd.collective_compute(
                "AllGather", mybir.AluOpType.bypass,
                replica_groups=[list(range(NCORES))],
                ins=[xn_bounce.opt()], outs=[xn_g.opt()])

            # ================= phase 1: dist + argmax =================
            # resident xT hi/lo: block k at cols [k*TC, (k+1)*TC)
            xTh_t = xtp.tile([P, (D // P) * TC], bf16)
            xTl_t = xtp.tile([P, (D // P) * TC], bf16)
            for k in range(D // P):
                nc.sync.dma_start(out=xTh_t[:, k * TC:(k + 1) * TC],
                                  in_=xTh_in[k * P:(k + 1) * P, :])
                nc.sync.dma_start(out=xTl_t[:, k * TC:(k + 1) * TC],
                                  in_=xTl_in[k * P:(k + 1) * P, :])

            cur_v = smp.tile([P, NTT], f32)   # running max per token tile
            cur_i = smp.tile([P, NTT], f32)   # running argmax (global code idx)

            for b in range(NB):
                # eT superblock: block k at cols [k*BLK, (k+1)*BLK)
                eTh_t = etp.tile([P, (D // P) * BLK], bf16, name="eTh_t",
                                 tag="eTh_t")
                eTl_t = etp.tile([P, (D // P) * BLK], bf16, name="eTl_t",
                                 tag="eTl_t")
                for k in range(D // P):
                    nc.sync.dma_start(
                        out=eTh_t[:, k * BLK:(k + 1) * BLK],
                        in_=eTh_in[k * P:(k + 1) * P, b * BLK:(b + 1) * BLK])
                    nc.sync.dma_start(
                        out=eTl_t[:, k * BLK:(k + 1) * BLK],
                        in_=eTl_in[k * P:(k + 1) * P, b * BLK:(b + 1) * BLK])

                for tt in range(NTT):
                    dst = dtp.tile([P, BLK], f32, name="dst", tag="dst")
                    for n in range(BLK // 512):
                        dps = pp.tile([P, 512], f32, name="dps", tag="dps")
                        nmm = (D // P) * 3
                        i = 0
                        for k in range(D // P):
                            for (lh, rh) in ((xTh_t, eTh_t), (xTh_t, eTl_t),
                                             (xTl_t, eTh_t)):
                                nc.tensor.matmul(
                                    out=dps[:],
                                    lhsT=lh[:, k * TC + tt * P:
                                            k * TC + (tt + 1) * P],
                                    rhs=rh[:, k * BLK + n * 512:
                                           k * BLK + (n + 1) * 512],
                                    start=(i == 0), stop=(i == nmm - 1),
                                )
                                i += 1
                        nc.scalar.mul(dst[:, n * 512:(n + 1) * 512], dps[:],
                                      inv_t[:, tt:tt + 1])
                    nc.sync.dma_start(
                        out=dist_out[tt * P:(tt + 1) * P,
                                     b * BLK:(b + 1) * BLK],
                        in_=dst[:])
                    # blockwise top-1
                    bmx = xp.tile([P, 8], f32, name="bmx", tag="bmx")
                    nc.vector.max(bmx[:], dst[:])
                    bix = xp.tile([P, 8], u32, name="bix", tag="bix")
                    nc.vector.max_index(bix[:], bmx[:], dst[:])
                    bixf = xp.tile([P, 1], f32, name="bixf", tag="bixf")
                    nc.vector.tensor_copy(bixf[:], bix[:, 0:1])
                    if b == 0:
                        nc.vector.tensor_copy(cur_v[:, tt:tt + 1], bmx[:, 0:1])
                        nc.vector.tensor_copy(cur_i[:, tt:tt + 1], bixf[:])
                    else:
                        gixf = xp.tile([P, 1], f32, name="gixf", tag="gixf")
                        nc.vector.tensor_scalar_add(gixf[:], bixf[:],
                                                    float(b * BLK))
                        win = xp.tile([P, 1], u8, name="win", tag="win")
                        nc.vector.tensor_tensor(
                            out=win[:], in0=bmx[:, 0:1], in1=cur_v[:, tt:tt + 1],
                            op=mybir.AluOpType.is_gt)
                        nc.vector.copy_predicated(cur_v[:, tt:tt + 1], win[:],
                                                  bmx[:, 0:1])
                        nc.vector.copy_predicated(cur_i[:, tt:tt + 1], win[:],
                                                  gixf[:])

            # ---- ind output + AllGather + quantize gather ----
            ind_t = smp.tile([P, NTT], i32)
            nc.vector.tensor_copy(ind_t[:], cur_i[:])
            for tt in range(NTT):
                nc.sync.dma_start(out=ind_bounce[tt * P:(tt + 1) * P, :],
                                  in_=ind_t[:, tt:tt + 1])
                nc.sync.dma_start(out=ind_out[tt * P:(tt + 1) * P, :],
                                  in_=ind_t[:, tt:tt + 1])
            nc.gpsimd.collective_compute(
                "AllGather", mybir.AluOpType.bypass,
                replica_groups=[list(range(NCORES))],
                ins=[ind_bounce.opt()], outs=[ind_g.opt()])
            for tt in range(NTT):
                qz_t = xp.tile([P, D], f32, name="qz_t", tag="qz_t")
                nc.gpsimd.indirect_dma_start(
                    out=qz_t[:], out_offset=None, in_=emb_in[:],
                    in_offset=bass.IndirectOffsetOnAxis(
                        ap=ind_t[:, tt:tt + 1], axis=0))
                nc.sync.dma_start(out=qz_out[tt * P:(tt + 1) * P, :],
                                  in_=qz_t[:])

            # ================= phase 2: EMA update (code shard) ============
            # gathered indices -> local (subtract rank offset), as f32
            indg_sb = smp.tile([P, NTG], i32)
            nc.sync.dma_start(
                out=indg_sb[:],
                in_=ind_g[:, 0:1].rearrange("(a p) o -> p (a o)", p=P))
            indl_f = smp.tile([P, NTG], f32)
            nc.vector.tensor_copy(indl_f[:], indg_sb[:])
            nc.vector.tensor_scalar_sub(indl_f[:], indl_f[:], offs_t[:, :1])

            bins_acc = smp.tile([P, CC], f32)
            nc.vector.memset(bins_acc[:], 0.0)

            NHALF = NCC // 2  # 4 code chunks per half-pass (4 PSUM banks)
            for half in range(2):
                es_ps = []
                for j in range(NHALF):
                    es_ps.append(esp.tile([P, D], f32, name=f"es{j}",
                                          tag=f"es{j}"))
                for tg in range(NTG):
                    xng_t = ph2.tile([P, D], f32r, name="xng_t", tag="xng_t")
                    nc.sync.dma_start(out=xng_t[:],
                                      in_=xn_g[tg * P:(tg + 1) * P, :])
                    oh_t = ph2.tile([P, CC], f32r, name="oh_t", tag="oh_t")
                    nc.vector.tensor_scalar(oh_t[:], iota_f[:],
                                            indl_f[:, tg:tg + 1], None,
                                            op0=mybir.AluOpType.is_equal)
                    if half == 0:
                        nc.vector.tensor_tensor(
                            out=bins_acc[:], in0=bins_acc[:], in1=oh_t[:],
                            op=mybir.AluOpType.add)
                    for j in range(NHALF):
                        cc = half * NHALF + j
                        nc.tensor.matmul(
                            out=es_ps[j][:],
                            lhsT=oh_t[:, cc * P:(cc + 1) * P],
                            rhs=xng_t[:],
                            start=(tg == 0), stop=(tg == NTG - 1),
                        )

                # ---- EMA epilogue for this half's code chunks ----
                for j in range(NHALF):
                    cc = half * NHALF + j
                    ea_t = ph2.tile([P, D], f32, name="ea_t", tag="ea_t")
                    nc.sync.dma_start(out=ea_t[:],
                                      in_=ea_in[cc * P:(cc + 1) * P, :])
                    nea_t = ph2.tile([P, D], f32, name="nea_t", tag="nea_t")
                    nc.scalar.mul(nea_t[:], ea_t[:], DECAY)
                    est = ph2.tile([P, D], f32, name="est", tag="est")
                    nc.scalar.mul(est[:], es_ps[j][:], 1.0 - DECAY)
                    nc.vector.tensor_tensor(out=nea_t[:], in0=nea_t[:],
                                            in1=est[:],
                                            op=mybir.AluOpType.add)
                    nc.sync.dma_start(out=nea_out[cc * P:(cc + 1) * P, :],
                                      in_=nea_t[:])
                    # new_embed = nea / max(||nea||, eps) (smoothing cancels)
                    sq2 = ph2.tile([P, D], f32, name="sq2", tag="sq2")
                    ssq2 = ph2.tile([P, 1], f32, name="ssq2", tag="ssq2")
                    nc.scalar.activation(sq2[:], nea_t[:], AFT.Square,
                                         accum_out=ssq2[:])
                    nrm2 = ph2.tile([P, 1], f32, name="nrm2", tag="nrm2")
                    nc.scalar.sqrt(nrm2[:], ssq2[:])
                    nc.vector.tensor_scalar_max(nrm2[:], nrm2[:], EPS_NORM)
                    inv2 = ph2.tile([P, 1], f32, name="inv2", tag="inv2")
                    nc.vector.reciprocal(inv2[:], nrm2[:])
                    nem_t = ph2.tile([P, D], f32, name="nem_t", tag="nem_t")
                    nc.scalar.mul(nem_t[:], nea_t[:], inv2[:, :1])
                    nc.sync.dma_start(out=nem_out[cc * P:(cc + 1) * P, :],
                                      in_=nem_t[:])

            # ---- bins partition-reduce + new_cluster_size ----
            bins_ps0 = bnp.tile([1, 512], f32, name="bins_ps0", tag="bins_ps0")
            bins_ps1 = bnp.tile([1, 512], f32, name="bins_ps1", tag="bins_ps1")
            nc.tensor.matmul(out=bins_ps0[:], lhsT=ones_f[:],
                             rhs=bins_acc[:, 0:512], start=True, stop=True)
            nc.tensor.matmul(out=bins_ps1[:], lhsT=ones_f[:],
                             rhs=bins_acc[:, 512:1024], start=True, stop=True)
            cs_t = smp.tile([1, CC], f32)
            nc.sync.dma_start(out=cs_t[:], in_=cs_in[:])
            ncs_t = smp.tile([1, CC], f32)
            nc.scalar.mul(ncs_t[:], cs_t[:], DECAY)
            binw = smp.tile([1, CC], f32)
            nc.scalar.mul(binw[:, 0:512], bins_ps0[:], 1.0 - DECAY)
            nc.scalar.mul(binw[:, 512:1024], bins_ps1[:], 1.0 - DECAY)
            nc.vector.tensor_tensor(out=ncs_t[:], in0=ncs_t[:], in1=binw[:],
                                    op=mybir.AluOpType.add)
            nc.sync.dma_start(out=ncs_out[:], in_=ncs_t[:])

    nc.compile()
    return nc


def _split_bf16(m):
    hi = m.astype(ml_dtypes.bfloat16)
    lo = (m - hi.astype(np.float32)).astype(ml_dtypes.bfloat16)
    return hi, lo


def kernel(x, embed, cluster_size, embed_avg):
    x = np.asarray(x, dtype=np.float32)
    embed = np.asarray(embed, dtype=np.float32)
    cluster_size = np.asarray(cluster_size, dtype=np.float32)
    embed_avg = np.asarray(embed_avg, dtype=np.float32)

    b, n, d = x.shape
    H, Cn, Dn = embed.shape
    assert (b * n, d, Cn) == (T, D, C), (x.shape, embed.shape)

    if "nc" not in _CACHE:
        _CACHE["nc"] = _build()
    nc = _CACHE["nc"]

    xf = x.reshape(T, D)
    e2 = embed[0]                       # (C, D)
    eT = np.ascontiguousarray(e2.T)     # (D, C)
    eTh, eTl = _split_bf16(eT)
    ea2 = embed_avg[0]                  # (C, D)
    cs2 = cluster_size                  # (1, C)

    in_maps = []
    for r in range(NCORES):
        xs = np.ascontiguousarray(xf[r * TC:(r + 1) * TC])
        xT = np.ascontiguousarray(xs.T)
        xTh, xTl = _split_bf16(xT)
        in_maps.append({
            "x_sh": xs,
            "xTh": xTh, "xTl": xTl,
            "eTh": eTh, "eTl": eTl,
            "emb": e2,
            "ea_sh": np.ascontiguousarray(ea2[r * CC:(r + 1) * CC]),
            "cs_sh": np.ascontiguousarray(cs2[:, r * CC:(r + 1) * CC]),
            "offs": np.full((P, 1), float(r * CC), dtype=np.float32),
        })

    res = run_bass_kernel_spmd(nc, in_maps, core_ids=list(range(NCORES)))
    rs = res.results

    dist = np.concatenate([rs[r]["dist_sh"] for r in range(NCORES)], axis=0)
    ind = np.concatenate([rs[r]["ind_sh"][:, 0] for r in range(NCORES)], axis=0)
    quant = np.concatenate([rs[r]["qz_sh"] for r in range(NCORES)], axis=0)
    ncs = np.concatenate([rs[r]["ncs_sh"] for r in range(NCORES)], axis=1)
    nea = np.concatenate([rs[r]["nea_sh"] for r in range(NCORES)], axis=0)
    nem = np.concatenate([rs[r]["nem_sh"] for r in range(NCORES)], axis=0)

    return (quant.reshape(b, n, d),
            ind.reshape(b, n).astype(np.int32),
            dist.reshape(b, n, C),
            ncs.reshape(1, C),
            nea.reshape(1, C, D),
            nem.reshape(1, C, D))


# revision 6
# speedup vs baseline: 1.6389x; 1.6389x over previous
"""CosineSimCodebook VQ kernel for 8 TRN2 NeuronCores.

Sharding:
  - dist/argmax/quantize: token-parallel (each core owns 1024 of 8192 tokens,
    computes dist against ALL 8192 codes -> local argmax is global argmax).
  - EMA update: code-parallel (each core owns 1024 of 8192 codes). The
    normalized tokens (xn, rounded to fp32r) and the argmax indices are
    AllGathered; each core then builds one-hot tiles for its code shard and
    accumulates embed_sum with a one-hot matmul over all tokens.

Precision:
  - dist matmul: host-split bf16 hi/lo 3-pass (x@e = xh@eh + xh@el + xl@eh),
    fp32 PSUM accumulation; row 1/||x|| scale fused into the PSUM->SBUF copy.
    Abs error ~1e-6 -- argmax matches fp32 reference.
  - embed_sum: one-hot (exact) fp32r matmul against fp32r xn (~3e-5 rel).
  - Laplace smoothing cancels inside l2norm (positive per-row scalar), so
    new_embed = new_embed_avg / max(||new_embed_avg||, 1e-6) directly.
"""
import sys

import numpy as np

if "/opt/trn_rl_repo" not in sys.path:
    sys.path.insert(0, "/opt/trn_rl_repo")

import ml_dtypes  # noqa: E402

import concourse.bass as bass  # noqa: E402
import concourse.tile as tile  # noqa: E402
from concourse import bacc, mybir  # noqa: E402
from concourse.bass_utils import run_bass_kernel_spmd  # noqa: E402
from concourse.masks import make_identity  # noqa: E402

P = 128
D = 512           # embedding dim
T = 8192          # total tokens (4*2048)
C = 8192          # codebook size
NCORES = 8
TC = T // NCORES  # tokens per core (1024)
CC = C // NCORES  # codes per core (1024)
NB = 4            # dist code superblocks
BLK = C // NB     # 2048 codes per superblock
NTT = TC // P     # token tiles per core (8)
NTG = T // P      # global token tiles (64)
NCC = CC // P     # code chunks per core (8)
DECAY = 0.8
EPS_NORM = 1e-6

f32 = mybir.dt.float32
f32r = mybir.dt.float32r
bf16 = mybir.dt.bfloat16
i32 = mybir.dt.int32
u32 = mybir.dt.uint32
u8 = mybir.dt.uint8

_CACHE = {}


def _build():
    nc = bacc.Bacc("TRN2", target_bir_lowering=False, debug=False,
                   enable_asserts=False, num_devices=NCORES)

    # ---- I/O (per core) ----
    x_in = nc.dram_tensor("x_sh", [TC, D], f32, kind="ExternalInput").ap()
    xTh_in = nc.dram_tensor("xTh", [D, TC], bf16, kind="ExternalInput").ap()
    xTl_in = nc.dram_tensor("xTl", [D, TC], bf16, kind="ExternalInput").ap()
    eTh_in = nc.dram_tensor("eTh", [D, C], bf16, kind="ExternalInput").ap()
    eTl_in = nc.dram_tensor("eTl", [D, C], bf16, kind="ExternalInput").ap()
    emb_in = nc.dram_tensor("emb", [C, D], f32, kind="ExternalInput").ap()
    ea_in = nc.dram_tensor("ea_sh", [CC, D], f32, kind="ExternalInput").ap()
    cs_in = nc.dram_tensor("cs_sh", [1, CC], f32, kind="ExternalInput").ap()
    offs_in = nc.dram_tensor("offs", [P, 1], f32, kind="ExternalInput").ap()

    dist_out = nc.dram_tensor("dist_sh", [TC, C], f32, kind="ExternalOutput").ap()
    ind_out = nc.dram_tensor("ind_sh", [TC, 1], i32, kind="ExternalOutput").ap()
    qz_out = nc.dram_tensor("qz_sh", [TC, D], f32, kind="ExternalOutput").ap()
    ncs_out = nc.dram_tensor("ncs_sh", [1, CC], f32, kind="ExternalOutput").ap()
    nea_out = nc.dram_tensor("nea_sh", [CC, D], f32, kind="ExternalOutput").ap()
    nem_out = nc.dram_tensor("nem_sh", [CC, D], f32, kind="ExternalOutput").ap()

    AFT = mybir.ActivationFunctionType

    with tile.TileContext(nc) as tc:
        with (
            tc.tile_pool(name="cst", bufs=1) as cst,
            tc.tile_pool(name="xp", bufs=2) as xp,          # x / xn streaming
            tc.tile_pool(name="xtp", bufs=1) as xtp,        # resident xT hi/lo
            tc.tile_pool(name="etp", bufs=2) as etp,        # eT superblocks
            tc.tile_pool(name="dtp", bufs=3) as dtp,        # dist sbuf blocks
            tc.tile_pool(name="smp", bufs=1) as smp,        # small persistent
            tc.tile_pool(name="ph2", bufs=3) as ph2,        # phase-2 streaming
            tc.tile_pool(name="pp", bufs=2, space="PSUM") as pp,
            tc.tile_pool(name="esp", bufs=1, space="PSUM") as esp,
            tc.tile_pool(name="bnp", bufs=1, space="PSUM") as bnp,
            tc.tile_pool(name="dram", bufs=1, space="DRAM") as dram,
        ):
            # ---- constants ----
            iota_i = cst.tile([P, CC], i32)
            nc.gpsimd.iota(iota_i[:], pattern=[[1, CC]], base=0,
                           channel_multiplier=0)
            iota_f = cst.tile([P, CC], f32)
            nc.vector.tensor_copy(iota_f[:], iota_i[:])
            ones_f = cst.tile([P, 1], f32)
            nc.vector.memset(ones_f[:], 1.0)
            offs_t = cst.tile([P, 1], f32)
            nc.sync.dma_start(out=offs_t[:], in_=offs_in[:])
            ident64 = cst.tile([64, 64], f32)
            make_identity(nc, ident64[:])

            # ---- collective buffers ----
            xn_bounce = dram.tile([TC, D], f32r, name="xn_bounce")
            xn_g = dram.tile([T, D], f32r, name="xn_g", addr_space="Shared")
            ind_bounce = dram.tile([TC, 1], i32, name="ind_bounce")
            ind_g = dram.tile([T, 1], i32, name="ind_g", addr_space="Shared")

            # ================= phase 0: norms + xn =================
            inv_t = smp.tile([P, NTT], f32)   # 1/max(||x_t||,eps), col=token tile
            for tt in range(NTT):
                x_t = xp.tile([P, D], f32, name="x_t", tag="x_t")
                nc.sync.dma_start(out=x_t[:], in_=x_in[tt * P:(tt + 1) * P, :])
                sq = xp.tile([P, D], f32, name="sq", tag="sq")
                ssq = xp.tile([P, 1], f32, name="ssq", tag="ssq")
                nc.scalar.activation(sq[:], x_t[:], AFT.Square, accum_out=ssq[:])
                nrm = xp.tile([P, 1], f32, name="nrm", tag="nrm")
                nc.scalar.sqrt(nrm[:], ssq[:])
                nc.vector.tensor_scalar_max(nrm[:], nrm[:], EPS_NORM)
                nc.vector.reciprocal(inv_t[:, tt:tt + 1], nrm[:])
                xn_t = xp.tile([P, D], f32r, name="xn_t", tag="xn_t")
                nc.scalar.mul(xn_t[:], x_t[:], inv_t[:, tt:tt + 1])
                nc.sync.dma_start(out=xn_bounce[tt * P:(tt + 1) * P, :],
                                  in_=xn_t[:])
            nc.gpsimd.collective_compute(
                "AllGather", mybir.AluOpType.bypass,
                replica_groups=[list(range(NCORES))],
                ins=[xn_bounce.opt()], outs=[xn_g.opt()])

            # ================= phase 1: dist + argmax =================
            # resident xT hi/lo: block k at cols [k*TC, (k+1)*TC)
            xTh_t = xtp.tile([P, (D // P) * TC], bf16)
            xTl_t = xtp.tile([P, (D // P) * TC], bf16)
            for k in range(D // P):
                nc.sync.dma_start(out=xTh_t[:, k * TC:(k + 1) * TC],
                                  in_=xTh_in[k * P:(k + 1) * P, :])
                nc.sync.dma_start(out=xTl_t[:, k * TC:(k + 1) * TC],
                                  in_=xTl_in[k * P:(k + 1) * P, :])

            cur_v = smp.tile([P, NTT], f32)   # running max per token tile
            cur_i = smp.tile([P, NTT], f32)   # running argmax (global code idx)

            for b in range(NB):
                # eT superblock: block k at cols [k*BLK, (k+1)*BLK)
                eTh_t = etp.tile([P, (D // P) * BLK], bf16, name="eTh_t",
                                 tag="eTh_t")
                eTl_t = etp.tile([P, (D // P) * BLK], bf16, name="eTl_t",
                                 tag="eTl_t")
                for k in range(D // P):
                    nc.sync.dma_start(
                        out=eTh_t[:, k * BLK:(k + 1) * BLK],
                        in_=eTh_in[k * P:(k + 1) * P, b * BLK:(b + 1) * BLK])
                    nc.sync.dma_start(
                        out=eTl_t[:, k * BLK:(k + 1) * BLK],
                        in_=eTl_in[k * P:(k + 1) * P, b * BLK:(b + 1) * BLK])

                for tt in range(NTT):
                    dst = dtp.tile([P, BLK], f32, name="dst", tag="dst")
                    for n in range(BLK // 512):
                        dps = pp.tile([P, 512], f32, name="dps", tag="dps")
                        nmm = (D // P) * 3
                        i = 0
                        for k in range(D // P):
                            for (lh, rh) in ((xTh_t, eTh_t), (xTh_t, eTl_t),
                                             (xTl_t, eTh_t)):
                                nc.tensor.matmul(
                                    out=dps[:],
                                    lhsT=lh[:, k * TC + tt * P:
                                            k * TC + (tt + 1) * P],
                                    rhs=rh[:, k * BLK + n * 512:
                                           k * BLK + (n + 1) * 512],
                                    start=(i == 0), stop=(i == nmm - 1),
                                )
                                i += 1
                        nc.scalar.mul(dst[:, n * 512:(n + 1) * 512], dps[:],
                                      inv_t[:, tt:tt + 1])
                    nc.sync.dma_start(
                        out=dist_out[tt * P:(tt + 1) * P,
                                     b * BLK:(b + 1) * BLK],
                        in_=dst[:])
                    # blockwise top-1
                    bmx = xp.tile([P, 8], f32, name="bmx", tag="bmx")
                    nc.vector.max(bmx[:], dst[:])
                    bix = xp.tile([P, 8], u32, name="bix", tag="bix")
                    nc.vector.max_index(bix[:], bmx[:], dst[:])
                    bixf = xp.tile([P, 1], f32, name="bixf", tag="bixf")
                    nc.vector.tensor_copy(bixf[:], bix[:, 0:1])
                    if b == 0:
                        nc.vector.tensor_copy(cur_v[:, tt:tt + 1], bmx[:, 0:1])
                        nc.vector.tensor_copy(cur_i[:, tt:tt + 1], bixf[:])
                    else:
                        gixf = xp.tile([P, 1], f32, name="gixf", tag="gixf")
                        nc.vector.tensor_scalar_add(gixf[:], bixf[:],
                                                    float(b * BLK))
                        win = xp.tile([P, 1], u8, name="win", tag="win")
                        nc.vector.tensor_tensor(
                            out=win[:], in0=bmx[:, 0:1], in1=cur_v[:, tt:tt + 1],
                            op=mybir.AluOpType.is_gt)
                        nc.vector.copy_predicated(cur_v[:, tt:tt + 1], win[:],
                                                  bmx[:, 0:1])
                        nc.vector.copy_predicated(cur_i[:, tt:tt + 1], win[:],
                                                  gixf[:])

            # ---- ind output + AllGather + quantize gather ----
            ind_t = smp.tile([P, NTT], i32)
            nc.vector.tensor_copy(ind_t[:], cur_i[:])
            for tt in range(NTT):
                nc.sync.dma_start(out=ind_bounce[tt * P:(tt + 1) * P, :],
                                  in_=ind_t[:, tt:tt + 1])
                nc.sync.dma_start(out=ind_out[tt * P:(tt + 1) * P, :],
                                  in_=ind_t[:, tt:tt + 1])
            nc.gpsimd.collective_compute(
                "AllGather", mybir.AluOpType.bypass,
                replica_groups=[list(range(NCORES))],
                ins=[ind_bounce.opt()], outs=[ind_g.opt()])
            for tt in range(NTT):
                qz_t = xp.tile([P, D], f32, name="qz_t", tag="qz_t")
                nc.gpsimd.indirect_dma_start(
                    out=qz_t[:], out_offset=None, in_=emb_in[:],
                    in_offset=bass.IndirectOffsetOnAxis(
                        ap=ind_t[:, tt:tt + 1], axis=0))
                nc.sync.dma_start(out=qz_out[tt * P:(tt + 1) * P, :],
                                  in_=qz_t[:])

            # ================= phase 2: EMA update (code shard) ============
            # gathered indices -> local (subtract rank offset), as f32
            indg_sb = smp.tile([NTG, P], i32)
            nc.sync.dma_start(
                out=indg_sb[:],
                in_=ind_g[:, 0:1].rearrange("(a p) o -> a (p o)", p=P))
            indg_f = smp.tile([NTG, P], f32)
            nc.vector.tensor_copy(indg_f[:], indg_sb[:])
            tp_ps = pp.tile([P, NTG], f32, name="tp_ps", tag="dps")
            nc.tensor.transpose(out=tp_ps[:], in_=indg_f[:],
                                identity=ident64[:])
            indl_f = smp.tile([P, NTG], f32)
            nc.vector.tensor_scalar(indl_f[:], tp_ps[:], offs_t[:, :1], None,
                                    op0=mybir.AluOpType.subtract)

            bins_acc = smp.tile([P, CC], f32)
            nc.vector.memset(bins_acc[:], 0.0)

            NHALF = NCC // 2  # 4 code chunks per half-pass (4 PSUM banks)
            for half in range(2):
                es_ps = []
                for j in range(NHALF):
                    es_ps.append(esp.tile([P, D], f32, name=f"es{j}",
                                          tag=f"es{j}"))
                for tg in range(NTG):
                    xng_t = ph2.tile([P, D], f32r, name="xng_t", tag="xng_t")
                    nc.sync.dma_start(out=xng_t[:],
                                      in_=xn_g[tg * P:(tg + 1) * P, :])
                    oh_t = ph2.tile([P, CC], f32r, name="oh_t", tag="oh_t")
                    nc.vector.tensor_scalar(oh_t[:], iota_f[:],
                                            indl_f[:, tg:tg + 1], None,
                                            op0=mybir.AluOpType.is_equal)
                    if half == 0:
                        nc.vector.tensor_tensor(
                            out=bins_acc[:], in0=bins_acc[:], in1=oh_t[:],
                            op=mybir.AluOpType.add)
                    for j in range(NHALF):
                        cc = half * NHALF + j
                        nc.tensor.matmul(
                            out=es_ps[j][:],
                            lhsT=oh_t[:, cc * P:(cc + 1) * P],
                            rhs=xng_t[:],
                            start=(tg == 0), stop=(tg == NTG - 1),
                        )

                # ---- EMA epilogue for this half's code chunks ----
                for j in range(NHALF):
                    cc = half * NHALF + j
                    ea_t = ph2.tile([P, D], f32, name="ea_t", tag="ea_t")
                    nc.sync.dma_start(out=ea_t[:],
                                      in_=ea_in[cc * P:(cc + 1) * P, :])
                    nea_t = ph2.tile([P, D], f32, name="nea_t", tag="nea_t")
                    nc.scalar.mul(nea_t[:], ea_t[:], DECAY)
                    est = ph2.tile([P, D], f32, name="est", tag="est")
                    nc.scalar.mul(est[:], es_ps[j][:], 1.0 - DECAY)
                    nc.vector.tensor_tensor(out=nea_t[:], in0=nea_t[:],
                                            in1=est[:],
                                            op=mybir.AluOpType.add)
                    nc.sync.dma_start(out=nea_out[cc * P:(cc + 1) * P, :],
                                      in_=nea_t[:])
                    # new_embed = nea / max(||nea||, eps) (smoothing cancels)
                    sq2 = ph2.tile([P, D], f32, name="sq2", tag="sq2")
                    ssq2 = ph2.tile([P, 1], f32, name="ssq2", tag="ssq2")
                    nc.scalar.activation(sq2[:], nea_t[:], AFT.Square,
                                         accum_out=ssq2[:])
                    nrm2 = ph2.tile([P, 1], f32, name="nrm2", tag="nrm2")
                    nc.scalar.sqrt(nrm2[:], ssq2[:])
                    nc.vector.tensor_scalar_max(nrm2[:], nrm2[:], EPS_NORM)
                    inv2 = ph2.tile([P, 1], f32, name="inv2", tag="inv2")
                    nc.vector.reciprocal(inv2[:], nrm2[:])
                    nem_t = ph2.tile([P, D], f32, name="nem_t", tag="nem_t")
                    nc.scalar.mul(nem_t[:], nea_t[:], inv2[:, :1])
                    nc.sync.dma_start(out=nem_out[cc * P:(cc + 1) * P, :],
                                      in_=nem_t[:])

            # ---- bins partition-reduce + new_cluster_size ----
            bins_ps0 = bnp.tile([1, 512], f32, name="bins_ps0", tag="bins_ps0")
            bins_ps1 = bnp.tile([1, 512], f32, name="bins_ps1", tag="bins_ps1")
            nc.tensor.matmul(out=bins_ps0[:], lhsT=ones_f[:],
                             rhs=bins_acc[:, 0:512], start=True, stop=True)
            nc.tensor.matmul(out=bins_ps1[:], lhsT=ones_f[:],
                             rhs=bins_acc[:, 512:1024], start=True, stop=True)
            cs_t = smp.tile([1, CC], f32)
            nc.sync.dma_start(out=cs_t[:], in_=cs_in[:])
            ncs_t = smp.tile([1, CC], f32)
            nc.scalar.mul(ncs_t[:], cs_t[:], DECAY)
            binw = smp.tile([1, CC], f32)
            nc.scalar.mul(binw[:, 0:512], bins_ps0[:], 1.0 - DECAY)
            nc.scalar.mul(binw[:, 512:1024], bins_ps1[:], 1.0 - DECAY)
            nc.vector.tensor_tensor(out=ncs_t[:], in0=ncs_t[:], in1=binw[:],
                                    op=mybir.AluOpType.add)
            nc.sync.dma_start(out=ncs_out[:], in_=ncs_t[:])

    nc.compile()
    return nc


def _split_bf16(m):
    hi = m.astype(ml_dtypes.bfloat16)
    lo = (m - hi.astype(np.float32)).astype(ml_dtypes.bfloat16)
    return hi, lo


def kernel(x, embed, cluster_size, embed_avg):
    x = np.asarray(x, dtype=np.float32)
    embed = np.asarray(embed, dtype=np.float32)
    cluster_size = np.asarray(cluster_size, dtype=np.float32)
    embed_avg = np.asarray(embed_avg, dtype=np.float32)

    b, n, d = x.shape
    H, Cn, Dn = embed.shape
    assert (b * n, d, Cn) == (T, D, C), (x.shape, embed.shape)

    if "nc" not in _CACHE:
        _CACHE["nc"] = _build()
    nc = _CACHE["nc"]

    xf = x.reshape(T, D)
    e2 = embed[0]                       # (C, D)
    eT = np.ascontiguousarray(e2.T)     # (D, C)
    eTh, eTl = _split_bf16(eT)
    ea2 = embed_avg[0]                  # (C, D)
    cs2 = cluster_size                  # (1, C)

    in_maps = []
    for r in range(NCORES):
        xs = np.ascontiguousarray(xf[r * TC:(r + 1) * TC])
        xT = np.ascontiguousarray(xs.T)
        xTh, xTl = _split_bf16(xT)
        in_maps.append({
            "x_sh": xs,
            "xTh": xTh, "xTl": xTl,
            "eTh": eTh, "eTl": eTl,
            "emb": e2,
            "ea_sh": np.ascontiguousarray(ea2[r * CC:(r + 1) * CC]),
            "cs_sh": np.ascontiguousarray(cs2[:, r * CC:(r + 1) * CC]),
            "offs": np.full((P, 1), float(r * CC), dtype=np.float32),
        })

    res = run_bass_kernel_spmd(nc, in_maps, core_ids=list(range(NCORES)))
    rs = res.results

    dist = np.concatenate([rs[r]["dist_sh"] for r in range(NCORES)], axis=0)
    ind = np.concatenate([rs[r]["ind_sh"][:, 0] for r in range(NCORES)], axis=0)
    quant = np.concatenate([rs[r]["qz_sh"] for r in range(NCORES)], axis=0)
    ncs = np.concatenate([rs[r]["ncs_sh"] for r in range(NCORES)], axis=1)
    nea = np.concatenate([rs[r]["nea_sh"] for r in range(NCORES)], axis=0)
    nem = np.concatenate([rs[r]["nem_sh"] for r in range(NCORES)], axis=0)

    return (quant.reshape(b, n, d),
            ind.reshape(b, n).astype(np.int32),
            dist.reshape(b, n, C),
            ncs.reshape(1, C),
            nea.reshape(1, C, D),
            nem.reshape(1, C, D))
